# revision 11
# baseline (speedup 1.0000x reference)
"""Trainium2 Bass kernel for nn_CCELoss (calibration-histogram loss), v2.

Sharding: data-parallel over image rows, 8 NeuronCores, 128 rows each.

Per-core layout: logits as [114 = 6 pixel-groups x 19 classes, F=45056]
(group g covers core-flat pixels [g*F, (g+1)*F)). NPIX=262144 valid pixels;
the tail of group 5 (tiles 9,10) is excluded from folds entirely (partitions
[95:114) unwritten there), so no pad corrections are needed.

Per 4096-pixel tile:
  ACT  e = exp(l)                  fp32 -> fp16
  PE   Z[g,n] = sum_c e[(g,c),n]   (block-diag ones matmul, fp16 -> PSUM f32)
  DMA  reshape Z [6,2048] -> [96,128] pixel-major (x2 halves)
  ACT  m = ln(Z)  on [96,256]      (cheap: 256 cols instead of 4096)
  DVE  d* = l* - m                 (pixel-major [96,256], true-class channel)
  DMA  m [96,256] -> lt[114:120]   (m joins the logit tile as 6 extra rows)
  PE   d = DM.T @ [l; m; l*]       (fp32 matmul: d = l - m[g], d* = l* - m, PSUM)
  ACT  p = exp(d)                  PSUM -> SBUF fp16
  19 folds on p (fp16, fp32 accumulation), one op each:
    counts N_i = sum [p > i/10]            (is_gt / add-reduce)
    S'_i   = sum max(p, i/10) - 4096*i/10  (max / add-reduce, scalar2 post-add)
    split DVE (fast 2-byte mode) / ACT (Relu bias, Sign) / GPSIMD
End: pstar = exp(dstar) on [96, 2816], DMA out (host bins accuracy histogram).
Host: decode S/N -> conf/cnt hists, bin p* -> acc hist, loss formula.
"""

import numpy as np

import bass_rust
import concourse.bass as bass
from concourse import bacc
import concourse.mybir as mybir
import concourse.tile as tile
from concourse.vector_clock import ScopedClock
from concourse.bass_utils import run_bass_kernel_spmd

F32 = mybir.dt.float32
F32R = mybir.dt.float32r
F16 = mybir.dt.float16
AF = mybir.ActivationFunctionType
ALU = mybir.AluOpType

# ---------------- problem geometry (hardcoded) ----------------
C = 19
NB = 10
H, W = 1024, 2048
NCORES = 8
ROWS = H // NCORES          # 128
NPIX = ROWS * W             # 262144 valid pixels per core
G = 6
P = G * C                   # 114 partitions of logits
PM = P + G                  # 120 partitions incl. m rows
PML = PM + G                # 126 partitions incl. l* rows
TILE_F = 4096
NT = 11
F = NT * TILE_F             # 45056
VALID_J5 = NPIX - 5 * F     # 36864 valid pixels in group 5
PAD_TILE0 = VALID_J5 // TILE_F  # = 9; tiles 9,10 have group 5 all-pad

THR = [np.float32(i / 10.0) for i in range(10)]


def _fp16_floor_thr():
    out = [np.float32(0.0)]
    for i in range(1, 10):
        t = i / 10.0
        t16 = np.float16(t)
        if float(t16) > t:
            t16 = np.nextafter(t16, np.float16(0.0))
        out.append(np.float32(t16))
    return out


# largest fp16 <= i/10: identical compare sets on the fp16 p grid, and
# max(p, t) / accumulated values stay exact through the fp16 scratch cast.
THR16 = _fp16_floor_thr()
NFOLD = 19                  # 9 counts (i=1..9) + 10 conf (i=0..9)
MM_CHUNK = 512

# fold engine assignment (tensor_scalar is not a legal GPSIMD opcode)
DVE_FOLDS = [("cnt", i) for i in range(1, 10)] + [("conf", i) for i in range(0, 8)]
ACT_FOLDS = [("conf", 8), ("conf", 9)]
NPAIR = NT // 2 + NT % 2    # fold groups: (0,1)..(8,9),(10)

_BUILD_CACHE = {}


def _patch_tile_drain():
    """walrus rejects drains with >1 sync wait; split the tile-exit drain."""
    if getattr(tile.TileContext, "_drain_patched", False):
        return

    def _drain_and_barrier(self, tick_clock, wait_clock):
        drain_inst = self.nc.sync.drain()
        wait_clock.add_sem_waits(
            drain_inst.ins, ScopedClock({None: tick_clock.global_clock})
        )
        si = drain_inst.ins.sync_info
        if si is not None and len(si.on_wait) > 1:
            waits = list(si.on_wait)
            ups = list(si.on_update)
            drain_inst.ins.sync_info = mybir.SyncInfo(on_wait=waits[:1], on_update=[])
            last = drain_inst
            for i in range(1, len(waits)):
                last = self.nc.sync.drain()
                last.ins.sync_info = mybir.SyncInfo(on_wait=waits[i:i + 1], on_update=[])
            if ups:
                lw = list(last.ins.sync_info.on_wait) if last.ins.sync_info else []
                last.ins.sync_info = mybir.SyncInfo(on_wait=lw, on_update=ups)
        self.nc.all_engine_barrier()
        assert self.sems is not None
        popped = self.nc._tile_sem_poison_stack.pop()
        assert popped is self._sem_poison
        self.nc.clear_and_free_semaphores(list(self.sems.allocated().values()))
        self.nc.all_engine_barrier()

    tile.TileContext._drain_and_barrier = _drain_and_barrier
    tile.TileContext._drain_patched = True


def _fold_slot(kind, i):
    # per-tile slot layout: counts i=1..9 -> 0..8; conf i=0..9 -> 9..18
    return (i - 1) if kind == "cnt" else (9 + i)


def build_nc():
    _patch_tile_drain()
    nc = bacc.Bacc()

    # const APs for ACT fold biases
    for kind, i in ACT_FOLDS:
        v = float(-THR16[i])
        if (F32, v) not in nc.const_aps.aps:
            tns = nc.alloc_sbuf_tensor(f"const-b{i}", [128, 1], F32)
            nc.gpsimd.memset(tns.ap(), v)
            nc.const_aps.aps[(F32, v)] = tns.ap()
    nc.all_engine_barrier()

    lg = nc.declare_dram_parameter("lg", [C, NPIX], F32, isOutput=False)
    lstar = nc.declare_dram_parameter("lstar", [G, F], F32, isOutput=False)
    bd16 = nc.declare_dram_parameter("bd16", [P, G], F16, isOutput=False)
    dmat = nc.declare_dram_parameter("dmat", [PML, PM], F32, isOutput=False)
    folds_out = nc.declare_dram_parameter("folds", [P, NPAIR * NFOLD], F32, isOutput=True)
    pstar_out = nc.declare_dram_parameter("pstar", [G, F], F16, isOutput=True)

    with tile.TileContext(nc) as tc:
        with (
            tc.tile_pool(name="const", bufs=1) as constp,
            tc.tile_pool(name="lt", bufs=2) as lp,
            tc.tile_pool(name="et", bufs=2) as ep,
            tc.tile_pool(name="pt", bufs=2) as pp,
            tc.tile_pool(name="mt", bufs=2) as mp,
            tc.tile_pool(name="acc", bufs=1) as accp,
            tc.tile_pool(name="zpsum", bufs=1, space="PSUM") as zp,
            tc.tile_pool(name="dpsum", bufs=1, space="PSUM") as dp,
        ):
            bd_sb = constp.tile([P, G], F16)
            nc.gpsimd.dma_start(out=bd_sb[:], in_=bd16[:])
            dm_sb = constp.tile([PML, PM], F32)
            nc.gpsimd.dma_start(out=dm_sb[:], in_=dmat[:])

            foldacc = accp.tile([P, NPAIR * NFOLD], F32)
            nc.gpsimd.memset(foldacc[:], 0.0)
            scr_dve = accp.tile([P, 2 * TILE_F], F16)
            scr_act = accp.tile([P, 2 * TILE_F], F16)

            for t in range(NT):
                pad = t >= PAD_TILE0
                ng = G - 1 if pad else G
                Pr = C * ng

                # ---- load logits tile rows [0:Pr] ----
                lt = lp.tile([PML, TILE_F], F32)
                if pad:
                    # pad columns get logits [0, -80 x18] on the group-5 rows
                    # -> p = [1, 0 x18] exactly; folded uniformly, corrected on
                    # host. (engine partition offsets must be 32-aligned; the
                    # DMA below overwrites rows [64:95) with real logits.)
                    nc.gpsimd.memset(lt[64:96, :], 0.0)
                    nc.gpsimd.memset(lt[96:P, :], -80.0)
                base = lg[:, t * TILE_F:(t + 1) * TILE_F]
                src3 = bass_rust.AP(
                    tensor=base.tensor, offset=base.offset,
                    ap=[[F, ng]] + list(base.ap))
                nc.gpsimd.dma_start(out=lt[0:Pr, :], in_=src3)
                # l* rows [120:126]
                nc.gpsimd.dma_start(
                    out=lt[PM:PML, :],
                    in_=lstar[:, t * TILE_F:(t + 1) * TILE_F])

                # ---- e = exp(l) -> fp16 ----
                et = ep.tile([P, TILE_F], F16)
                nc.scalar.activation(et[:], lt[0:P, :], AF.Exp)

                # ---- Z per half -> mt via Ln, then DMA into lt[114:120] ----
                mt = mp.tile([G, TILE_F], F32)
                for h in range(2):
                    zps = zp.tile([G, 2048], F32)
                    for q in range(4):
                        c0 = h * 2048 + q * MM_CHUNK
                        nc.tensor.matmul(
                            zps[:, q * MM_CHUNK:(q + 1) * MM_CHUNK],
                            bd_sb[:],
                            et[:, c0:c0 + MM_CHUNK],
                            start=True, stop=True,
                        )
                    nc.scalar.activation(
                        mt[:, h * 2048:(h + 1) * 2048], zps[:], AF.Ln)
                nc.gpsimd.dma_start(out=lt[P:PM, :], in_=mt[:])

                # ---- d = DM.T @ [l; m; l*] (fp32), p/p* = exp(d) ----
                if t % 2 == 0:
                    pt = pp.tile([PM, 2 * TILE_F], F16)
                pc0 = (t % 2) * TILE_F
                for h in range(2):
                    dps = dp.tile([PM, 2048], F32)
                    for s in range(4):
                        c0 = h * 2048 + s * MM_CHUNK
                        nc.tensor.matmul(
                            dps[:, s * MM_CHUNK:(s + 1) * MM_CHUNK],
                            dm_sb[:],
                            lt[:, c0:c0 + MM_CHUNK],
                            start=True, stop=True,
                        )
                    nc.scalar.activation(
                        pt[:, pc0 + h * 2048:pc0 + (h + 1) * 2048], dps[:], AF.Exp)

                # ---- p* rows out ----
                nc.gpsimd.dma_start(
                    out=pstar_out[:, t * TILE_F:(t + 1) * TILE_F],
                    in_=pt[P:PM, pc0:pc0 + TILE_F])

                # ---- folds: on tile pairs (and the final single tile) ----
                if t % 2 == 1 or t == NT - 1:
                    wf = TILE_F if t == NT - 1 and t % 2 == 0 else 2 * TILE_F
                    grp = t // 2
                    fb = foldacc[:, grp * NFOLD:(grp + 1) * NFOLD]
                    for kind, i in DVE_FOLDS:
                        s = _fold_slot(kind, i)
                        op0 = ALU.is_gt if kind == "cnt" else ALU.max
                        nc.vector.tensor_scalar(
                            scr_dve[:, 0:wf], pt[0:P, 0:wf], float(THR16[i]), None,
                            op0, ALU.add, accum_out=fb[:, s:s + 1])
                    for kind, i in ACT_FOLDS:
                        s = _fold_slot(kind, i)
                        nc.scalar.activation(
                            scr_act[:, 0:wf], pt[0:P, 0:wf], AF.Relu,
                            bias=float(-THR16[i]), accum_out=fb[:, s:s + 1])

            # ---- end phase ----
            nc.gpsimd.dma_start(out=folds_out[:], in_=foldacc[:])

    nc.finalize()
    return nc


def _make_consts():
    bd = np.zeros((P, G), np.float16)
    dm = np.zeros((PML, PM), np.float32)
    for g in range(G):
        bd[C * g:C * (g + 1), g] = 1.0
    for k in range(P):
        dm[k, k] = 1.0
        dm[P + k // C, k] = -1.0
    for g in range(G):
        dm[PM + g, P + g] = 1.0
        dm[P + g, P + g] = -1.0
    return bd, dm


def _shard_host(output: np.ndarray, target: np.ndarray):
    o = np.ascontiguousarray(output[0])          # [19, 1024, 2048]
    t = np.ascontiguousarray(target[0])          # [1024, 2048]
    lstar_full = np.take_along_axis(o, t[None], axis=0)[0]
    bd, dm = _make_consts()

    NPAD = G * F - NPIX
    in_maps = []
    for core in range(NCORES):
        r0 = core * ROWS
        lgc = np.ascontiguousarray(o[:, r0:r0 + ROWS, :].reshape(C, NPIX))
        ls = lstar_full[r0:r0 + ROWS, :].reshape(-1)
        ls = np.concatenate([ls, np.zeros(NPAD, np.float32)]).reshape(G, F)
        in_maps.append({
            "lg": lgc, "lstar": np.ascontiguousarray(ls),
            "bd16": bd, "dmat": dm,
        })
    return in_maps


def _decode_and_loss(results, target: np.ndarray):
    conf = np.zeros((C, NB), np.float64)
    cnt = np.zeros((C, NB), np.float64)
    acc = np.zeros((C, NB), np.float64)
    tgrid = np.array([float(t) for t in THR16], dtype=np.float64)
    act_slots = {i for (k, i) in ACT_FOLDS}

    PADCOLS = 2 * TILE_F      # 8192 pad cols per class-row (tiles 9,10)
    for core in range(NCORES):
        folds = results[core]["folds"].astype(np.float64)
        folds = folds.reshape(P, NPAIR, NFOLD).sum(axis=1)        # [114, 19]
        folds = folds.reshape(G, C, NFOLD).sum(axis=0)            # [C, 19]
        Ncnt = folds[:, 0:9]                                      # [C, 9] i=1..9
        M = folds[:, 9:19]                                        # [C, 10]
        # pad corrections: pad columns contribute p=1 on class 0, p=0 on 1..18
        Ncnt[0, :] -= PADCOLS
        for i in range(10):
            if i in act_slots:        # ACT Relu fold: relu(1-t) on class 0
                M[0, i] -= PADCOLS * (1.0 - tgrid[i])
            else:                     # max fold: max(1,t)=1 cls0; max(0,t)=t rest
                M[0, i] -= PADCOLS * 1.0
                M[1:, i] -= PADCOLS * tgrid[i]
        Ni = np.concatenate(
            [np.full((C, 1), float(NPIX)), Ncnt], axis=1)            # [C, 10]
        # max-form conf folds accumulated sum(max(p,t)) over NPIX valid cols;
        # R = M - t*NPIX.  ACT Relu folds are already R.
        R = np.empty_like(M)
        for i in range(10):
            R[:, i] = M[:, i] if i in act_slots else M[:, i] - tgrid[i] * NPIX

        S = R + tgrid[None, :] * Ni              # S_i = sum p * [p > t_i]
        Snext = np.concatenate([S[:, 1:], np.zeros((C, 1))], axis=1)
        Nnext = np.concatenate([Ni[:, 1:], np.zeros((C, 1))], axis=1)
        conf += S - Snext
        cnt += Ni - Nnext

        r0 = core * ROWS
        ps = results[core]["pstar"].astype(np.float32).reshape(-1)[:NPIX]
        y = target[0, r0:r0 + ROWS, :].reshape(-1)
        b = np.clip(np.ceil(ps * np.float32(10.0)).astype(np.int32) - 1, 0, NB - 1)
        acc += np.bincount(y * NB + b, minlength=C * NB).reshape(C, NB)

    EPS = 1e-13
    avg_acc = acc / (cnt + EPS)
    avg_conf = conf / (cnt + EPS)
    loss = np.sum((avg_acc - avg_conf) ** 2 * (cnt / cnt.sum()))
    return np.float32(loss), (conf, cnt, acc)


def kernel(output: np.ndarray, target: np.ndarray) -> np.ndarray:
    output = np.asarray(output, np.float32)
    target = np.asarray(target, np.int32)
    if "nc" not in _BUILD_CACHE:
        _BUILD_CACHE["nc"] = build_nc()
    nc = _BUILD_CACHE["nc"]
    in_maps = _shard_host(output, target)
    res = run_bass_kernel_spmd(nc, in_maps, list(range(NCORES)))
    loss, _ = _decode_and_loss(res.results, target)
    return np.float32(loss)


# revision 17
# speedup vs baseline: 1.0390x; 1.0390x over previous
"""Trainium2 Bass kernel for nn_CCELoss (calibration-histogram loss), v2.

Sharding: data-parallel over image rows, 8 NeuronCores, 128 rows each.

Per-core layout: logits as [114 = 6 pixel-groups x 19 classes, F=45056]
(group g covers core-flat pixels [g*F, (g+1)*F)). NPIX=262144 valid pixels;
the tail of group 5 (tiles 9,10) is excluded from folds entirely (partitions
[95:114) unwritten there), so no pad corrections are needed.

Per 4096-pixel tile:
  ACT  e = exp(l)                  fp32 -> fp16
  PE   Z[g,n] = sum_c e[(g,c),n]   (block-diag ones matmul, fp16 -> PSUM f32)
  DMA  reshape Z [6,2048] -> [96,128] pixel-major (x2 halves)
  ACT  m = ln(Z)  on [96,256]      (cheap: 256 cols instead of 4096)
  DVE  d* = l* - m                 (pixel-major [96,256], true-class channel)
  DMA  m [96,256] -> lt[114:120]   (m joins the logit tile as 6 extra rows)
  PE   d = DM.T @ [l; m; l*]       (fp32 matmul: d = l - m[g], d* = l* - m, PSUM)
  ACT  p = exp(d)                  PSUM -> SBUF fp16
  19 folds on p (fp16, fp32 accumulation), one op each:
    counts N_i = sum [p > i/10]            (is_gt / add-reduce)
    S'_i   = sum max(p, i/10) - 4096*i/10  (max / add-reduce, scalar2 post-add)
    split DVE (fast 2-byte mode) / ACT (Relu bias, Sign) / GPSIMD
End: pstar = exp(dstar) on [96, 2816], DMA out (host bins accuracy histogram).
Host: decode S/N -> conf/cnt hists, bin p* -> acc hist, loss formula.
"""

import numpy as np

import bass_rust
import concourse.bass as bass
from concourse import bacc
import concourse.mybir as mybir
import concourse.tile as tile
from concourse.vector_clock import ScopedClock
from concourse.bass_utils import run_bass_kernel_spmd

F32 = mybir.dt.float32
F32R = mybir.dt.float32r
F16 = mybir.dt.float16
AF = mybir.ActivationFunctionType
ALU = mybir.AluOpType

# ---------------- problem geometry (hardcoded) ----------------
C = 19
NB = 10
H, W = 1024, 2048
NCORES = 8
ROWS = H // NCORES          # 128
NPIX = ROWS * W             # 262144 valid pixels per core
G = 6
P = G * C                   # 114 partitions of logits
PM = P + G                  # 120 partitions incl. m rows
PML = PM + G                # 126 partitions incl. l* rows
TILE_F = 4096
NT = 11
F = NT * TILE_F             # 45056
VALID_J5 = NPIX - 5 * F     # 36864 valid pixels in group 5
PAD_TILE0 = VALID_J5 // TILE_F  # = 9; tiles 9,10 have group 5 all-pad

THR = [np.float32(i / 10.0) for i in range(10)]


def _fp16_floor_thr():
    out = [np.float32(0.0)]
    for i in range(1, 10):
        t = i / 10.0
        t16 = np.float16(t)
        if float(t16) > t:
            t16 = np.nextafter(t16, np.float16(0.0))
        out.append(np.float32(t16))
    return out


# largest fp16 <= i/10: identical compare sets on the fp16 p grid, and
# max(p, t) / accumulated values stay exact through the fp16 scratch cast.
THR16 = _fp16_floor_thr()
NFOLD = 19                  # 9 counts (i=1..9) + 10 conf (i=0..9)
MM_CHUNK = 512

# fold engine assignment (tensor_scalar is not a legal GPSIMD opcode)
N_ACT_FOLDS = 2             # conf folds 9, 8, ... assigned to ACT
GRPW = 2                    # tiles per fold group
PT_BUFS = 2
DP_BUFS = 1
DP_COLS = 2048
MT_BUFS = 2
ACT_FOLDS = [("conf", 9 - k) for k in range(N_ACT_FOLDS)]
DVE_FOLDS = ([("cnt", i) for i in range(1, 10)]
             + [("conf", i) for i in range(0, 10 - N_ACT_FOLDS)])
NPAIR = (NT + GRPW - 1) // GRPW   # fold groups

_BUILD_CACHE = {}


def _patch_tile_drain():
    """walrus rejects drains with >1 sync wait; split the tile-exit drain."""
    if getattr(tile.TileContext, "_drain_patched", False):
        return

    def _drain_and_barrier(self, tick_clock, wait_clock):
        drain_inst = self.nc.sync.drain()
        wait_clock.add_sem_waits(
            drain_inst.ins, ScopedClock({None: tick_clock.global_clock})
        )
        si = drain_inst.ins.sync_info
        if si is not None and len(si.on_wait) > 1:
            waits = list(si.on_wait)
            ups = list(si.on_update)
            drain_inst.ins.sync_info = mybir.SyncInfo(on_wait=waits[:1], on_update=[])
            last = drain_inst
            for i in range(1, len(waits)):
                last = self.nc.sync.drain()
                last.ins.sync_info = mybir.SyncInfo(on_wait=waits[i:i + 1], on_update=[])
            if ups:
                lw = list(last.ins.sync_info.on_wait) if last.ins.sync_info else []
                last.ins.sync_info = mybir.SyncInfo(on_wait=lw, on_update=ups)
        self.nc.all_engine_barrier()
        assert self.sems is not None
        popped = self.nc._tile_sem_poison_stack.pop()
        assert popped is self._sem_poison
        self.nc.clear_and_free_semaphores(list(self.sems.allocated().values()))
        self.nc.all_engine_barrier()

    tile.TileContext._drain_and_barrier = _drain_and_barrier
    tile.TileContext._drain_patched = True


def _fold_slot(kind, i):
    # per-tile slot layout: counts i=1..9 -> 0..8; conf i=0..9 -> 9..18
    return (i - 1) if kind == "cnt" else (9 + i)


def build_nc():
    _patch_tile_drain()
    nc = bacc.Bacc()

    # const APs for ACT fold biases
    for kind, i in ACT_FOLDS:
        v = float(-THR16[i])
        if (F32, v) not in nc.const_aps.aps:
            tns = nc.alloc_sbuf_tensor(f"const-b{i}", [128, 1], F32)
            nc.gpsimd.memset(tns.ap(), v)
            nc.const_aps.aps[(F32, v)] = tns.ap()
    nc.all_engine_barrier()

    lg = nc.declare_dram_parameter("lg", [C, NPIX], F32, isOutput=False)
    lstar = nc.declare_dram_parameter("lstar", [G, F], F32, isOutput=False)
    bd16 = nc.declare_dram_parameter("bd16", [P, G], F16, isOutput=False)
    dmat = nc.declare_dram_parameter("dmat", [PML, PM], F32, isOutput=False)
    folds_out = nc.declare_dram_parameter("folds", [P, NPAIR * NFOLD], F32, isOutput=True)
    pstar_out = nc.declare_dram_parameter("pstar", [G, F], F16, isOutput=True)

    with tile.TileContext(nc) as tc:
        with (
            tc.tile_pool(name="const", bufs=1) as constp,
            tc.tile_pool(name="lt", bufs=3) as lp,
            tc.tile_pool(name="et", bufs=2) as ep,
            tc.tile_pool(name="pt", bufs=PT_BUFS) as pp,
            tc.tile_pool(name="mt", bufs=MT_BUFS) as mp,
            tc.tile_pool(name="acc", bufs=1) as accp,
            tc.tile_pool(name="zpsum", bufs=1, space="PSUM") as zp,
            tc.tile_pool(name="dpsum", bufs=DP_BUFS, space="PSUM") as dp,
        ):
            bd_sb = constp.tile([P, G], F16)
            nc.gpsimd.dma_start(out=bd_sb[:], in_=bd16[:])
            dm_sb = constp.tile([PML, PM], F32)
            nc.gpsimd.dma_start(out=dm_sb[:], in_=dmat[:])

            foldacc = accp.tile([P, NPAIR * NFOLD], F32)
            nc.gpsimd.memset(foldacc[:], 0.0)
            scr_dve = accp.tile([P, GRPW * TILE_F], F16)
            scr_act = accp.tile([P, GRPW * TILE_F], F16)

            lts = {}
            pts = {}

            def st_load(t):
                # logits + l* rows of tile t (2-tile lookahead)
                pad = t >= PAD_TILE0
                ng = G - 1 if pad else G
                lt = lts[t] = lp.tile([PML, TILE_F], F32, name="lt")
                if pad:
                    # pad columns get logits [0, -80 x18] on the group-5 rows
                    # -> p = [1, 0 x18] exactly; folded uniformly, corrected on
                    # host. (engine partition offsets must be 32-aligned; the
                    # DMA below overwrites rows [64:95) with real logits.)
                    nc.gpsimd.memset(lt[64:96, :], 0.0)
                    nc.gpsimd.memset(lt[96:P, :], -80.0)
                base = lg[:, t * TILE_F:(t + 1) * TILE_F]
                src3 = bass_rust.AP(
                    tensor=base.tensor, offset=base.offset,
                    ap=[[F, ng]] + list(base.ap))
                nc.gpsimd.dma_start(out=lt[0:C * ng, :], in_=src3)
                nc.gpsimd.dma_start(
                    out=lt[PM:PML, :],
                    in_=lstar[:, t * TILE_F:(t + 1) * TILE_F])

            def st_norm(t):
                # e = exp(l); Z halves on PE; m = ln(Z) -> lt[114:120]
                lt = lts[t]
                et = ep.tile([P, TILE_F], F16)
                nc.scalar.activation(et[:], lt[0:P, :], AF.Exp)
                mt = mp.tile([G, TILE_F], F32)
                for h in range(2):
                    zps = zp.tile([G, 2048], F32)
                    for q in range(4):
                        c0 = h * 2048 + q * MM_CHUNK
                        nc.tensor.matmul(
                            zps[:, q * MM_CHUNK:(q + 1) * MM_CHUNK],
                            bd_sb[:],
                            et[:, c0:c0 + MM_CHUNK],
                            start=True, stop=True,
                        )
                    nc.scalar.activation(
                        mt[:, h * 2048:(h + 1) * 2048], zps[:], AF.Ln)
                nc.gpsimd.dma_start(out=lt[P:PM, :], in_=mt[:])

            def st_prob(t):
                # d = DM.T @ [l; m; l*] (fp32), p/p* = exp(d)
                lt = lts.pop(t)
                if t % GRPW == 0:
                    pts[t // GRPW] = pp.tile([PM, GRPW * TILE_F], F16, name="ptg")
                pt = pts[t // GRPW]
                pc0 = (t % GRPW) * TILE_F
                for h in range(TILE_F // DP_COLS):
                    dps = dp.tile([PM, DP_COLS], F32)
                    for s in range(DP_COLS // MM_CHUNK):
                        c0 = h * DP_COLS + s * MM_CHUNK
                        nc.tensor.matmul(
                            dps[:, s * MM_CHUNK:(s + 1) * MM_CHUNK],
                            dm_sb[:],
                            lt[:, c0:c0 + MM_CHUNK],
                            start=True, stop=True,
                        )
                    nc.scalar.activation(
                        pt[:, pc0 + h * DP_COLS:pc0 + (h + 1) * DP_COLS],
                        dps[:], AF.Exp)
                nc.gpsimd.dma_start(
                    out=pstar_out[:, t * TILE_F:(t + 1) * TILE_F],
                    in_=pt[P:PM, pc0:pc0 + TILE_F])

            def st_fold(t):
                # folds over the completed group ending at tile t
                wf = (t % GRPW + 1) * TILE_F
                grp = t // GRPW
                pt = pts.pop(grp)
                fb = foldacc[:, grp * NFOLD:(grp + 1) * NFOLD]
                for kind, i in DVE_FOLDS:
                    s = _fold_slot(kind, i)
                    op0 = ALU.is_gt if kind == "cnt" else ALU.max
                    nc.vector.tensor_scalar(
                        scr_dve[:, 0:wf], pt[0:P, 0:wf], float(THR16[i]), None,
                        op0, ALU.add, accum_out=fb[:, s:s + 1])
                for kind, i in ACT_FOLDS:
                    s = _fold_slot(kind, i)
                    nc.scalar.activation(
                        scr_act[:, 0:wf], pt[0:P, 0:wf], AF.Relu,
                        bias=float(-THR16[i]), accum_out=fb[:, s:s + 1])

            # software-pipelined schedule: loads 2 ahead, norm 1 ahead of prob
            st_load(0)
            st_load(1)
            st_norm(0)
            for t in range(NT):
                if t + 2 < NT:
                    st_load(t + 2)
                if t + 1 < NT:
                    st_norm(t + 1)
                st_prob(t)
                if t % GRPW == GRPW - 1 or t == NT - 1:
                    st_fold(t)

            # ---- end phase ----
            nc.gpsimd.dma_start(out=folds_out[:], in_=foldacc[:])

    nc.finalize()
    return nc


def _make_consts():
    bd = np.zeros((P, G), np.float16)
    dm = np.zeros((PML, PM), np.float32)
    for g in range(G):
        bd[C * g:C * (g + 1), g] = 1.0
    for k in range(P):
        dm[k, k] = 1.0
        dm[P + k // C, k] = -1.0
    for g in range(G):
        dm[PM + g, P + g] = 1.0
        dm[P + g, P + g] = -1.0
    return bd, dm


def _shard_host(output: np.ndarray, target: np.ndarray):
    o = np.ascontiguousarray(output[0])          # [19, 1024, 2048]
    t = np.ascontiguousarray(target[0])          # [1024, 2048]
    lstar_full = np.take_along_axis(o, t[None], axis=0)[0]
    bd, dm = _make_consts()

    NPAD = G * F - NPIX
    in_maps = []
    for core in range(NCORES):
        r0 = core * ROWS
        lgc = np.ascontiguousarray(o[:, r0:r0 + ROWS, :].reshape(C, NPIX))
        ls = lstar_full[r0:r0 + ROWS, :].reshape(-1)
        ls = np.concatenate([ls, np.zeros(NPAD, np.float32)]).reshape(G, F)
        in_maps.append({
            "lg": lgc, "lstar": np.ascontiguousarray(ls),
            "bd16": bd, "dmat": dm,
        })
    return in_maps


def _decode_and_loss(results, target: np.ndarray):
    conf = np.zeros((C, NB), np.float64)
    cnt = np.zeros((C, NB), np.float64)
    acc = np.zeros((C, NB), np.float64)
    tgrid = np.array([float(t) for t in THR16], dtype=np.float64)
    act_slots = {i for (k, i) in ACT_FOLDS}

    PADCOLS = 2 * TILE_F      # 8192 pad cols per class-row (tiles 9,10)
    for core in range(NCORES):
        folds = results[core]["folds"].astype(np.float64)
        folds = folds.reshape(P, NPAIR, NFOLD).sum(axis=1)        # [114, 19]
        folds = folds.reshape(G, C, NFOLD).sum(axis=0)            # [C, 19]
        Ncnt = folds[:, 0:9]                                      # [C, 9] i=1..9
        M = folds[:, 9:19]                                        # [C, 10]
        # pad corrections: pad columns contribute p=1 on class 0, p=0 on 1..18
        Ncnt[0, :] -= PADCOLS
        for i in range(10):
            if i in act_slots:        # ACT Relu fold: relu(1-t) on class 0
                M[0, i] -= PADCOLS * (1.0 - tgrid[i])
            else:                     # max fold: max(1,t)=1 cls0; max(0,t)=t rest
                M[0, i] -= PADCOLS * 1.0
                M[1:, i] -= PADCOLS * tgrid[i]
        Ni = np.concatenate(
            [np.full((C, 1), float(NPIX)), Ncnt], axis=1)            # [C, 10]
        # max-form conf folds accumulated sum(max(p,t)) over NPIX valid cols;
        # R = M - t*NPIX.  ACT Relu folds are already R.
        R = np.empty_like(M)
        for i in range(10):
            R[:, i] = M[:, i] if i in act_slots else M[:, i] - tgrid[i] * NPIX

        S = R + tgrid[None, :] * Ni              # S_i = sum p * [p > t_i]
        Snext = np.concatenate([S[:, 1:], np.zeros((C, 1))], axis=1)
        Nnext = np.concatenate([Ni[:, 1:], np.zeros((C, 1))], axis=1)
        conf += S - Snext
        cnt += Ni - Nnext

        r0 = core * ROWS
        ps = results[core]["pstar"].astype(np.float32).reshape(-1)[:NPIX]
        y = target[0, r0:r0 + ROWS, :].reshape(-1)
        b = np.clip(np.ceil(ps * np.float32(10.0)).astype(np.int32) - 1, 0, NB - 1)
        acc += np.bincount(y * NB + b, minlength=C * NB).reshape(C, NB)

    EPS = 1e-13
    avg_acc = acc / (cnt + EPS)
    avg_conf = conf / (cnt + EPS)
    loss = np.sum((avg_acc - avg_conf) ** 2 * (cnt / cnt.sum()))
    return np.float32(loss), (conf, cnt, acc)


def kernel(output: np.ndarray, target: np.ndarray) -> np.ndarray:
    output = np.asarray(output, np.float32)
    target = np.asarray(target, np.int32)
    if "nc" not in _BUILD_CACHE:
        _BUILD_CACHE["nc"] = build_nc()
    nc = _BUILD_CACHE["nc"]
    in_maps = _shard_host(output, target)
    res = run_bass_kernel_spmd(nc, in_maps, list(range(NCORES)))
    loss, _ = _decode_and_loss(res.results, target)
    return np.float32(loss)


# revision 18
# speedup vs baseline: 1.1304x; 1.0880x over previous
"""Trainium2 Bass kernel for nn_CCELoss (calibration-histogram loss), v2.

Sharding: data-parallel over image rows, 8 NeuronCores, 128 rows each.

Per-core layout: logits as [114 = 6 pixel-groups x 19 classes, F=45056]
(group g covers core-flat pixels [g*F, (g+1)*F)).  Pad columns (group-5 tail,
tiles 9-10) carry logits [0, -80 x18] so p = [1, 0 x18] exactly; they are
folded uniformly and corrected exactly on the host.

Software-pipelined per 4096-pixel tile (loads 2 tiles ahead, normalization 1
tile ahead of the probability stage):
  ACT  e = exp(l)                  fp32 -> fp16
  PE   Z[g,n] = sum_c e[(g,c),n]   (block-diag ones matmul, fp16 -> PSUM f32)
  ACT  m = ln(Z)                   [6,2048] x2, PSUM -> SBUF
  DMA  m -> lt[114:120]            (m joins the logit tile as 6 extra rows)
  PE   d = DM.T @ [l; m; l*]       (fp32 matmul: d = l - m[g], d* = l* - m;
                                    l* rows [120:126] give the true-class
                                    side channel for the accuracy histogram)
  ACT  p/p* = exp(d)               PSUM -> SBUF fp16
  19 folds on p (fp16 data, fp32 accumulation), one op each, thresholds
  t'_i = largest-fp16 <= i/10 (identical compare sets to i/10 on the fp16
  grid; max(p, t') stays exact through the fp16 scratch cast):
    counts N_i = sum [p > t'_i]        (DVE tensor_scalar is_gt, accum)
    M_i = sum max(p, t'_i)             (DVE tensor_scalar max, accum)
    R_9 = sum relu(p - t'_9)           (ACT Relu bias fold)
Host: R_i = M_i - t'_i*NPIX, S_i = R_i + t'_i*N_i, conf_i = S_i - S_{i+1},
cnt_i = N_i - N_{i+1}; p* binned against target for the accuracy histogram;
pad corrections; final loss formula in fp64.
"""

import numpy as np

import bass_rust
import concourse.bass as bass
from concourse import bacc
import concourse.mybir as mybir
import concourse.tile as tile
from concourse.vector_clock import ScopedClock
from concourse.bass_utils import run_bass_kernel_spmd

F32 = mybir.dt.float32
F32R = mybir.dt.float32r
F16 = mybir.dt.float16
AF = mybir.ActivationFunctionType
ALU = mybir.AluOpType

# ---------------- problem geometry (hardcoded) ----------------
C = 19
NB = 10
H, W = 1024, 2048
NCORES = 8
ROWS = H // NCORES          # 128
NPIX = ROWS * W             # 262144 valid pixels per core
G = 6
P = G * C                   # 114 partitions of logits
PM = P + G                  # 120 partitions incl. m rows
PML = PM + G                # 126 partitions incl. l* rows
TILE_F = 4096
NT = 11
F = NT * TILE_F             # 45056
VALID_J5 = NPIX - 5 * F     # 36864 valid pixels in group 5
PAD_TILE0 = VALID_J5 // TILE_F  # = 9; tiles 9,10 have group 5 all-pad

THR = [np.float32(i / 10.0) for i in range(10)]


def _fp16_floor_thr():
    out = [np.float32(0.0)]
    for i in range(1, 10):
        t = i / 10.0
        t16 = np.float16(t)
        if float(t16) > t:
            t16 = np.nextafter(t16, np.float16(0.0))
        out.append(np.float32(t16))
    return out


# largest fp16 <= i/10: identical compare sets on the fp16 p grid, and
# max(p, t) / accumulated values stay exact through the fp16 scratch cast.
THR16 = _fp16_floor_thr()
NFOLD = 19                  # 9 counts (i=1..9) + 10 conf (i=0..9)
MM_CHUNK = 512

# fold engine assignment (tensor_scalar is not a legal GPSIMD opcode)
N_ACT_FOLDS = 1             # conf folds 9, 8, ... assigned to ACT
GRPW = 1                    # tiles per fold group
PT_BUFS = 2
DP_BUFS = 1
DP_COLS = 2048
MT_BUFS = 2
ACT_FOLDS = [("conf", 9 - k) for k in range(N_ACT_FOLDS)]
DVE_FOLDS = ([("cnt", i) for i in range(1, 10)]
             + [("conf", i) for i in range(0, 10 - N_ACT_FOLDS)])
NPAIR = (NT + GRPW - 1) // GRPW   # fold groups

_BUILD_CACHE = {}


def _patch_tile_drain():
    """walrus rejects drains with >1 sync wait; split the tile-exit drain."""
    if getattr(tile.TileContext, "_drain_patched", False):
        return

    def _drain_and_barrier(self, tick_clock, wait_clock):
        drain_inst = self.nc.sync.drain()
        wait_clock.add_sem_waits(
            drain_inst.ins, ScopedClock({None: tick_clock.global_clock})
        )
        si = drain_inst.ins.sync_info
        if si is not None and len(si.on_wait) > 1:
            waits = list(si.on_wait)
            ups = list(si.on_update)
            drain_inst.ins.sync_info = mybir.SyncInfo(on_wait=waits[:1], on_update=[])
            last = drain_inst
            for i in range(1, len(waits)):
                last = self.nc.sync.drain()
                last.ins.sync_info = mybir.SyncInfo(on_wait=waits[i:i + 1], on_update=[])
            if ups:
                lw = list(last.ins.sync_info.on_wait) if last.ins.sync_info else []
                last.ins.sync_info = mybir.SyncInfo(on_wait=lw, on_update=ups)
        self.nc.all_engine_barrier()
        assert self.sems is not None
        popped = self.nc._tile_sem_poison_stack.pop()
        assert popped is self._sem_poison
        self.nc.clear_and_free_semaphores(list(self.sems.allocated().values()))
        self.nc.all_engine_barrier()

    tile.TileContext._drain_and_barrier = _drain_and_barrier
    tile.TileContext._drain_patched = True


def _fold_slot(kind, i):
    # per-tile slot layout: counts i=1..9 -> 0..8; conf i=0..9 -> 9..18
    return (i - 1) if kind == "cnt" else (9 + i)


def build_nc():
    _patch_tile_drain()
    nc = bacc.Bacc()

    # const APs for ACT fold biases
    for kind, i in ACT_FOLDS:
        v = float(-THR16[i])
        if (F32, v) not in nc.const_aps.aps:
            tns = nc.alloc_sbuf_tensor(f"const-b{i}", [128, 1], F32)
            nc.gpsimd.memset(tns.ap(), v)
            nc.const_aps.aps[(F32, v)] = tns.ap()
    nc.all_engine_barrier()

    lg = nc.declare_dram_parameter("lg", [C, NPIX], F32, isOutput=False)
    lstar = nc.declare_dram_parameter("lstar", [G, F], F32, isOutput=False)
    bd16 = nc.declare_dram_parameter("bd16", [P, G], F16, isOutput=False)
    dmat = nc.declare_dram_parameter("dmat", [PML, PM], F32, isOutput=False)
    folds_out = nc.declare_dram_parameter("folds", [P, NPAIR * NFOLD], F32, isOutput=True)
    pstar_out = nc.declare_dram_parameter("pstar", [G, F], F16, isOutput=True)

    with tile.TileContext(nc) as tc:
        with (
            tc.tile_pool(name="const", bufs=1) as constp,
            tc.tile_pool(name="lt", bufs=3) as lp,
            tc.tile_pool(name="et", bufs=2) as ep,
            tc.tile_pool(name="pt", bufs=PT_BUFS) as pp,
            tc.tile_pool(name="mt", bufs=MT_BUFS) as mp,
            tc.tile_pool(name="acc", bufs=1) as accp,
            tc.tile_pool(name="zpsum", bufs=1, space="PSUM") as zp,
            tc.tile_pool(name="dpsum", bufs=DP_BUFS, space="PSUM") as dp,
        ):
            bd_sb = constp.tile([P, G], F16)
            nc.gpsimd.dma_start(out=bd_sb[:], in_=bd16[:])
            dm_sb = constp.tile([PML, PM], F32)
            nc.gpsimd.dma_start(out=dm_sb[:], in_=dmat[:])

            foldacc = accp.tile([P, NPAIR * NFOLD], F32)
            nc.gpsimd.memset(foldacc[:], 0.0)
            scr_dve = accp.tile([P, GRPW * TILE_F], F16)
            scr_act = accp.tile([P, GRPW * TILE_F], F16)

            lts = {}
            pts = {}

            def st_load(t):
                # logits + l* rows of tile t (2-tile lookahead)
                pad = t >= PAD_TILE0
                ng = G - 1 if pad else G
                lt = lts[t] = lp.tile([PML, TILE_F], F32, name="lt")
                if pad:
                    # pad columns get logits [0, -80 x18] on the group-5 rows
                    # -> p = [1, 0 x18] exactly; folded uniformly, corrected on
                    # host. (engine partition offsets must be 32-aligned; the
                    # DMA below overwrites rows [64:95) with real logits.)
                    nc.gpsimd.memset(lt[64:96, :], 0.0)
                    nc.gpsimd.memset(lt[96:P, :], -80.0)
                base = lg[:, t * TILE_F:(t + 1) * TILE_F]
                src3 = bass_rust.AP(
                    tensor=base.tensor, offset=base.offset,
                    ap=[[F, ng]] + list(base.ap))
                nc.gpsimd.dma_start(out=lt[0:C * ng, :], in_=src3)
                nc.gpsimd.dma_start(
                    out=lt[PM:PML, :],
                    in_=lstar[:, t * TILE_F:(t + 1) * TILE_F])

            def st_norm(t):
                # e = exp(l); Z halves on PE; m = ln(Z) -> lt[114:120]
                lt = lts[t]
                et = ep.tile([P, TILE_F], F16)
                nc.scalar.activation(et[:], lt[0:P, :], AF.Exp)
                mt = mp.tile([G, TILE_F], F32)
                for h in range(2):
                    zps = zp.tile([G, 2048], F32)
                    for q in range(4):
                        c0 = h * 2048 + q * MM_CHUNK
                        nc.tensor.matmul(
                            zps[:, q * MM_CHUNK:(q + 1) * MM_CHUNK],
                            bd_sb[:],
                            et[:, c0:c0 + MM_CHUNK],
                            start=True, stop=True,
                        )
                    nc.scalar.activation(
                        mt[:, h * 2048:(h + 1) * 2048], zps[:], AF.Ln)
                nc.gpsimd.dma_start(out=lt[P:PM, :], in_=mt[:])

            def st_prob(t):
                # d = DM.T @ [l; m; l*] (fp32), p/p* = exp(d)
                lt = lts.pop(t)
                if t % GRPW == 0:
                    pts[t // GRPW] = pp.tile([PM, GRPW * TILE_F], F16, name="ptg")
                pt = pts[t // GRPW]
                pc0 = (t % GRPW) * TILE_F
                for h in range(TILE_F // DP_COLS):
                    dps = dp.tile([PM, DP_COLS], F32)
                    for s in range(DP_COLS // MM_CHUNK):
                        c0 = h * DP_COLS + s * MM_CHUNK
                        nc.tensor.matmul(
                            dps[:, s * MM_CHUNK:(s + 1) * MM_CHUNK],
                            dm_sb[:],
                            lt[:, c0:c0 + MM_CHUNK],
                            start=True, stop=True,
                        )
                    nc.scalar.activation(
                        pt[:, pc0 + h * DP_COLS:pc0 + (h + 1) * DP_COLS],
                        dps[:], AF.Exp)
                nc.gpsimd.dma_start(
                    out=pstar_out[:, t * TILE_F:(t + 1) * TILE_F],
                    in_=pt[P:PM, pc0:pc0 + TILE_F])

            def st_fold(t):
                # folds over the completed group ending at tile t
                wf = (t % GRPW + 1) * TILE_F
                grp = t // GRPW
                pt = pts.pop(grp)
                fb = foldacc[:, grp * NFOLD:(grp + 1) * NFOLD]
                for kind, i in DVE_FOLDS:
                    s = _fold_slot(kind, i)
                    op0 = ALU.is_gt if kind == "cnt" else ALU.max
                    nc.vector.tensor_scalar(
                        scr_dve[:, 0:wf], pt[0:P, 0:wf], float(THR16[i]), None,
                        op0, ALU.add, accum_out=fb[:, s:s + 1])
                for kind, i in ACT_FOLDS:
                    s = _fold_slot(kind, i)
                    nc.scalar.activation(
                        scr_act[:, 0:wf], pt[0:P, 0:wf], AF.Relu,
                        bias=float(-THR16[i]), accum_out=fb[:, s:s + 1])

            # software-pipelined schedule: loads 2 ahead, norm 1 ahead of prob
            st_load(0)
            st_load(1)
            st_norm(0)
            for t in range(NT):
                if t + 2 < NT:
                    st_load(t + 2)
                if t + 1 < NT:
                    st_norm(t + 1)
                st_prob(t)
                if t % GRPW == GRPW - 1 or t == NT - 1:
                    st_fold(t)

            # ---- end phase ----
            nc.gpsimd.dma_start(out=folds_out[:], in_=foldacc[:])

    nc.finalize()
    return nc


def _make_consts():
    bd = np.zeros((P, G), np.float16)
    dm = np.zeros((PML, PM), np.float32)
    for g in range(G):
        bd[C * g:C * (g + 1), g] = 1.0
    for k in range(P):
        dm[k, k] = 1.0
        dm[P + k // C, k] = -1.0
    for g in range(G):
        dm[PM + g, P + g] = 1.0
        dm[P + g, P + g] = -1.0
    return bd, dm


def _shard_host(output: np.ndarray, target: np.ndarray):
    o = np.ascontiguousarray(output[0])          # [19, 1024, 2048]
    t = np.ascontiguousarray(target[0])          # [1024, 2048]
    lstar_full = np.take_along_axis(o, t[None], axis=0)[0]
    bd, dm = _make_consts()

    NPAD = G * F - NPIX
    in_maps = []
    for core in range(NCORES):
        r0 = core * ROWS
        lgc = np.ascontiguousarray(o[:, r0:r0 + ROWS, :].reshape(C, NPIX))
        ls = lstar_full[r0:r0 + ROWS, :].reshape(-1)
        ls = np.concatenate([ls, np.zeros(NPAD, np.float32)]).reshape(G, F)
        in_maps.append({
            "lg": lgc, "lstar": np.ascontiguousarray(ls),
            "bd16": bd, "dmat": dm,
        })
    return in_maps


def _decode_and_loss(results, target: np.ndarray):
    conf = np.zeros((C, NB), np.float64)
    cnt = np.zeros((C, NB), np.float64)
    acc = np.zeros((C, NB), np.float64)
    tgrid = np.array([float(t) for t in THR16], dtype=np.float64)
    act_slots = {i for (k, i) in ACT_FOLDS}

    PADCOLS = 2 * TILE_F      # 8192 pad cols per class-row (tiles 9,10)
    for core in range(NCORES):
        folds = results[core]["folds"].astype(np.float64)
        folds = folds.reshape(P, NPAIR, NFOLD).sum(axis=1)        # [114, 19]
        folds = folds.reshape(G, C, NFOLD).sum(axis=0)            # [C, 19]
        Ncnt = folds[:, 0:9]                                      # [C, 9] i=1..9
        M = folds[:, 9:19]                                        # [C, 10]
        # pad corrections: pad columns contribute p=1 on class 0, p=0 on 1..18
        Ncnt[0, :] -= PADCOLS
        for i in range(10):
            if i in act_slots:        # ACT Relu fold: relu(1-t) on class 0
                M[0, i] -= PADCOLS * (1.0 - tgrid[i])
            else:                     # max fold: max(1,t)=1 cls0; max(0,t)=t rest
                M[0, i] -= PADCOLS * 1.0
                M[1:, i] -= PADCOLS * tgrid[i]
        Ni = np.concatenate(
            [np.full((C, 1), float(NPIX)), Ncnt], axis=1)            # [C, 10]
        # max-form conf folds accumulated sum(max(p,t)) over NPIX valid cols;
        # R = M - t*NPIX.  ACT Relu folds are already R.
        R = np.empty_like(M)
        for i in range(10):
            R[:, i] = M[:, i] if i in act_slots else M[:, i] - tgrid[i] * NPIX

        S = R + tgrid[None, :] * Ni              # S_i = sum p * [p > t_i]
        Snext = np.concatenate([S[:, 1:], np.zeros((C, 1))], axis=1)
        Nnext = np.concatenate([Ni[:, 1:], np.zeros((C, 1))], axis=1)
        conf += S - Snext
        cnt += Ni - Nnext

        r0 = core * ROWS
        ps = results[core]["pstar"].astype(np.float32).reshape(-1)[:NPIX]
        y = target[0, r0:r0 + ROWS, :].reshape(-1)
        b = np.clip(np.ceil(ps * np.float32(10.0)).astype(np.int32) - 1, 0, NB - 1)
        acc += np.bincount(y * NB + b, minlength=C * NB).reshape(C, NB)

    EPS = 1e-13
    avg_acc = acc / (cnt + EPS)
    avg_conf = conf / (cnt + EPS)
    loss = np.sum((avg_acc - avg_conf) ** 2 * (cnt / cnt.sum()))
    return np.float32(loss), (conf, cnt, acc)


def kernel(output: np.ndarray, target: np.ndarray) -> np.ndarray:
    output = np.asarray(output, np.float32)
    target = np.asarray(target, np.int32)
    if "nc" not in _BUILD_CACHE:
        _BUILD_CACHE["nc"] = build_nc()
    nc = _BUILD_CACHE["nc"]
    in_maps = _shard_host(output, target)
    res = run_bass_kernel_spmd(nc, in_maps, list(range(NCORES)))
    loss, _ = _decode_and_loss(res.results, target)
    return np.float32(loss)
